# revision 24
# baseline (speedup 1.0000x reference)
"""DeepseekV3 top-k router kernel for 8x Trainium2 NeuronCores.

Strategy (v3):
  - Token dim (8192) sharded 8 ways; router weight replicated per core.
  - logits = hidden @ W.T decomposed as  xh*wh  +  (xl*w + xh*wl):
      * P1: one 256-wide fp16 matmul per k-tile: stationary xh16, moving wh16
        -> psum_w.
      * cross: one fp8e4 DoubleRow matmul per k-tile with stationary
        [xl*2^16 | x*2^5] and moving [w*2^11 | (w-wh16)*2^22]; both slot
        products land at scale 2^27 in one psum_d.
    Bias folded exactly via a 57th k-tile (b16 through P1; b_res*2^22 through
    the DoubleRow slot1 against a 32.0 stationary row).
    The fp8 copy of x (slot1) is derived ON-CHIP by the ACT engine from the
    already-resident xh16 tiles (scale-32 copy), so x ships as just
    fp16 hi (2B) + fp8 lo (1B).
    Empirically (fixed-seed inputs) this flips 4 near-tie rows of 8192
    (idx frobenius ~8e-3, weight frobenius ~2e-4) - well inside the gate.
  - stage2 per 128-token tile on DVE/ACT: psum combine, sigmoid, grouped
    top-2-sum via reduce_max + match_replace + reduce_max, top-4 groups,
    masked top-8 via max_with_indices, eb-bias un-correction via one-hot
    match rows + one reduction, normalization.
  - DMA descriptors spread across sync/scalar/gpsimd queues.
"""

import numpy as np
import ml_dtypes

import concourse.bacc as bacc
import concourse.mybir as mybir
from concourse.tile import TileContext
from concourse import bass_utils

H = 7168
E = 256
T = 8192
NCORES = 8
TLOC = T // NCORES          # 1024 tokens per core
MT = 128                    # tokens per tile (PSUM partition dim)
NM = TLOC // MT             # 8 token tiles per core
KT = H // 128               # 56 contraction tiles
KP = KT + 1                 # +1 bias tile
TOP_K = 8
N_GROUP = 8
TOPK_GROUP = 4
EG = E // N_GROUP           # 32 experts per group
SCALE = 2.5
SX8 = 2.0 ** 16             # xl -> fp8 pre-scale
SW8 = 2.0 ** 11             # w  -> fp8 pre-scale
SXH8 = 2.0 ** 5             # x  -> fp8 pre-scale (slot1 stationary)
SWL8 = 2.0 ** 22            # wl -> fp8 pre-scale (slot1 moving)
DSC = 1.0 / (SX8 * SW8)     # = 2^-27; slot1 product matches: 2^5 * 2^22
NEG = -1e30

f32 = mybir.dt.float32
f16 = mybir.dt.float16
f8 = mybir.dt.float8e4
u32 = mybir.dt.uint32
i32 = mybir.dt.int32
AOT = mybir.AluOpType
ACTF = mybir.ActivationFunctionType
DR = mybir.MatmulPerfMode.DoubleRow
NPF8 = ml_dtypes.float8_e4m3

_PROG = None

# per-m chunk sizes in k-tiles (sum = KP = 57); shared by xh and xdr
XH_CH0 = [2, 6, 8, 8, 8, 8, 8, 9]     # m == 0 (fine-grained startup)
XH_CH = [4, 4, 8, 8, 8, 8, 8, 9]      # m > 0
# fp16 wh chunk sizes (k-tiles, sum = 57)
W_CH = [2, 2, 4, 4, 8, 8, 8, 8, 8, 5]
# fp8 wdr chunk sizes (k-tiles, sum = 57)
W8_CH = [2, 2, 4, 4, 8, 8, 8, 8, 8, 5]


def _offs(ch):
    return [sum(ch[:i]) for i in range(len(ch))]


def _build():
    nc = bacc.Bacc(trn_type="TRN2")
    XHD = nc.dram_tensor("xh", [NM, 128, KP, MT], f16, kind="ExternalInput")
    XLD = nc.dram_tensor("xl", [NM, 128, KP, MT], f8, kind="ExternalInput")
    WHD = nc.dram_tensor("wh", [128, KP, E], f16, kind="ExternalInput")
    WDR = nc.dram_tensor("wdr", [128, KP, 2 * E], f8, kind="ExternalInput")
    C = nc.dram_tensor("c", [128, 2 * E], f32, kind="ExternalInput")
    OIDX = nc.dram_tensor("oidx", [TLOC, TOP_K], i32, kind="ExternalOutput")
    OW = nc.dram_tensor("ow", [TLOC, TOP_K], f32, kind="ExternalOutput")

    with TileContext(nc) as tc:
        with (
            tc.tile_pool(name="const", bufs=1) as cpool,
            tc.tile_pool(name="xh0", bufs=1) as xh0pool,
            tc.tile_pool(name="xd0", bufs=1) as xd0pool,
            tc.tile_pool(name="xh", bufs=3) as xhpool,
            tc.tile_pool(name="xd", bufs=3) as xdpool,
            tc.tile_pool(name="s2", bufs=2) as s2,
            tc.tile_pool(name="scrp", bufs=2) as scrp,
            tc.tile_pool(name="pw", bufs=2, space="PSUM") as ppw,
            tc.tile_pool(name="pd", bufs=2, space="PSUM") as ppd,
            tc.tile_pool(name="pw1", bufs=1, space="PSUM") as ppw1,
            tc.tile_pool(name="pd1", bufs=1, space="PSUM") as ppd1,
            tc.tile_pool(name="pw2", bufs=1, space="PSUM") as ppw2,
            tc.tile_pool(name="pd2", bufs=1, space="PSUM") as ppd2,
        ):
            # ---- resident constants / weights (scalar-engine DMA queue) ----
            c_sb = cpool.tile([128, 2 * E], f32, name="c_sb")
            nc.sync.dma_start(c_sb[:, :], C[:, :])
            eb_rep = c_sb[:, E:2 * E]

            w_off = _offs(W_CH)
            w8_off = _offs(W8_CH)
            w_sbs = [None] * len(W_CH)
            w8_sbs = [None] * len(W8_CH)
            merged = []
            wi = w8i = 0
            for k in range(KP):
                while wi < len(W_CH) and w_off[wi] <= k:
                    merged.append(("w", wi)); wi += 1
                while w8i < len(W8_CH) and w8_off[w8i] <= k:
                    merged.append(("w8", w8i)); w8i += 1
            for kind, i in merged:
                if kind == "w":
                    nk = W_CH[i]
                    t = cpool.tile([128, nk * E], f16, name=f"w_sb{i}")
                    nc.sync.dma_start(
                        t.rearrange("p (k e) -> p k e", k=nk),
                        WHD[:, w_off[i]:w_off[i] + nk, :])
                    w_sbs[i] = t
                else:
                    nk = W8_CH[i]
                    t = cpool.tile([128, nk * 2 * E], f8, name=f"w8_sb{i}")
                    nc.sync.dma_start(
                        t.rearrange("p (k e) -> p k e", k=nk),
                        WDR[:, w8_off[i]:w8_off[i] + nk, :])
                    w8_sbs[i] = t

            wmap = []
            for ci, n in enumerate(W_CH):
                wmap += [(ci, j) for j in range(n)]
            w8map = []
            for ci, n in enumerate(W8_CH):
                w8map += [(ci, j) for j in range(n)]

            # ---- x loads: xh fp16 (sync q) + xdr fp8 assembled from a
            #      strided DMA (slot0=xl8, gpsimd q) and an ACT cast of the
            #      xh tile into slot1 (= x*2^5 in fp8) ----
            def load_m(m, offs, ch):
                """Issue all of tile-m's DMAs first, then the fp8 casts, so
                descriptor issue is never blocked behind a data wait."""
                hp = xh0pool if m == 0 else xhpool
                dp = xd0pool if m == 0 else xdpool
                tiles = []
                for i in range(len(ch)):
                    nk = ch[i]
                    xh_t = hp.tile([128, nk * MT], f16, tag=f"xh{m == 0}{i}",
                                   name=f"xh{i}_{m}")
                    nc.scalar.dma_start(
                        xh_t.rearrange("p (k t) -> p k t", k=nk),
                        XHD[m, :, offs[i]:offs[i] + nk, :])
                    xd_t = dp.tile([128, 2 * nk * MT], f8, tag=f"xd{m == 0}{i}",
                                   name=f"xd{i}_{m}")
                    xd4 = xd_t.rearrange("p (a k t) -> p a k t", a=2, k=nk)
                    nc.gpsimd.dma_start(xd4[:, 0, :, :],
                                        XLD[m, :, offs[i]:offs[i] + nk, :])
                    tiles.append((xh_t, xd_t))
                for i in range(len(ch)):
                    nk = ch[i]
                    xh_t, xd_t = tiles[i]
                    xd4 = xd_t.rearrange("p (a k t) -> p a k t", a=2, k=nk)
                    nc.scalar.activation(
                        xd4[:, 1, :, :],
                        xh_t.rearrange("p (k t) -> p k t", k=nk),
                        ACTF.Copy, scale=SXH8)
                return tiles

            xh0_off, xh_off = _offs(XH_CH0), _offs(XH_CH)

            def xh_maps(ch):
                mp = []
                for ci, n in enumerate(ch):
                    mp += [(ci, j) for j in range(n)]
                return mp

            x_pre = {}
            for m, (cho, cof) in ((0, (XH_CH0, xh0_off)), (1, (XH_CH, xh_off)),
                                  (2, (XH_CH, xh_off))):
                x_pre[m] = (load_m(m, cof, cho), xh_maps(cho))

            def stage2(m, pw, pd):
                t0 = s2.tile([128, E], f32, tag="t0", name=f"t0_{m}")
                nc.vector.tensor_scalar(t0[:, :], pd[:, :], DSC, None,
                                        op0=AOT.mult)
                lg = s2.tile([128, E], f32, tag="lg", name=f"lg_{m}")
                nc.vector.scalar_tensor_tensor(
                    lg[:, :], pw[:, :], 1.0, t0[:, :],
                    op0=AOT.mult, op1=AOT.add)
                s = s2.tile([128, E], f32, tag="s", name=f"s_{m}")
                nc.scalar.activation(s[:, :], lg[:, :], ACTF.Sigmoid)
                sfc = s2.tile([128, E], f32, tag="sfc", name=f"sfc_{m}")
                nc.gpsimd.tensor_add(sfc[:, :], s[:, :], eb_rep)

                sfc3 = sfc.rearrange("p (g c) -> p g c", c=EG)
                r1 = s2.tile([128, N_GROUP], f32, tag="r1", name=f"r1_{m}")
                nc.vector.tensor_reduce(r1[:, :], sfc3,
                                        axis=mybir.AxisListType.X, op=AOT.max)
                mr = s2.tile([128, E], f32, tag="mr", name=f"mr_{m}")
                nc.vector.match_replace(mr[:, :], r1[:, :], sfc[:, :], NEG)
                r2 = s2.tile([128, N_GROUP], f32, tag="r2", name=f"r2_{m}")
                nc.vector.tensor_reduce(r2[:, :],
                                        mr.rearrange("p (g c) -> p g c", c=EG),
                                        axis=mybir.AxisListType.X, op=AOT.max)
                gs = s2.tile([128, N_GROUP], f32, tag="gs", name=f"gs_{m}")
                nc.gpsimd.tensor_add(gs[:, :], r1[:, :], r2[:, :])
                g8 = s2.tile([128, 8], f32, tag="g8", name=f"g8_{m}")
                nc.vector.max(out=g8[:, :], in_=gs[:, :])
                gmask = s2.tile([128, N_GROUP], f32, tag="gmask", name=f"gmask_{m}")
                nc.vector.tensor_scalar(gmask[:, :], gs[:, :],
                                        g8[:, TOPK_GROUP - 1:TOPK_GROUP], None,
                                        op0=AOT.is_ge)
                masked = s2.tile([128, E], f32, tag="masked", name=f"masked_{m}")
                nc.gpsimd.tensor_mul(
                    masked.rearrange("p (g c) -> p g c", c=EG), sfc3,
                    gmask.unsqueeze(2).to_broadcast([128, N_GROUP, EG]))
                m8 = s2.tile([128, 8], f32, tag="m8", name=f"m8_{m}")
                i8 = s2.tile([128, 8], u32, tag="i8", name=f"i8_{m}")
                nc.vector.max_with_indices(out_max=m8[:, :], out_indices=i8[:, :],
                                           in_=masked[:, :])
                nc.sync.dma_start(OIDX[m * MT:(m + 1) * MT, :], i8.bitcast(i32))

                # eb[idx_j] via one-hot match rows; halves split across the
                # Vector and Pool engines to halve the chain latency
                eb8 = s2.tile([128, 8], f32, tag="eb8", name=f"eb8_{m}")
                scr = scrp.tile([128, TOP_K * E], f16, tag="scr", name=f"scr_{m}")
                for j in range(TOP_K):
                    nc.vector.scalar_tensor_tensor(
                        scr[:, j * E:(j + 1) * E], sfc[:, :], m8[:, j:j + 1],
                        eb_rep, op0=AOT.is_equal, op1=AOT.mult)
                # fp16 binary add-tree on the Pool engine: 256 -> 1 per row
                ta = scrp.tile([128, TOP_K * 128], f16, tag="ta", name=f"ta_{m}")
                tb = scrp.tile([128, TOP_K * 64], f16, tag="tb", name=f"tb_{m}")
                w = E
                cur = scr
                dsts = [ta, tb, ta, tb, ta, tb, ta]
                for lvl in range(7):
                    w //= 2
                    dst = dsts[lvl]
                    c3 = cur.rearrange("p (j e) -> p j e", j=TOP_K)
                    d3 = dst.rearrange("p (j e) -> p j e", j=TOP_K)                         if dst.shape[1] == TOP_K * w else                         dst.rearrange("p (j e) -> p j e", j=TOP_K)[:, :, :w]
                    nc.gpsimd.tensor_add(d3[:, :, 0:w], c3[:, :, 0:w],
                                         c3[:, :, w:2 * w])
                    cur = dst
                c3 = cur.rearrange("p (j e) -> p j e", j=TOP_K)
                nc.gpsimd.tensor_add(eb8.unsqueeze(2), c3[:, :, 0:1],
                                     c3[:, :, 1:2])
                w8v = s2.tile([128, 8], f32, tag="w8v", name=f"w8v_{m}")
                nc.vector.tensor_tensor(w8v[:, :], m8[:, :], eb8[:, :],
                                        op=AOT.subtract)
                rs = s2.tile([128, 1], f32, tag="rs", name=f"rs_{m}")
                nc.vector.tensor_reduce(rs[:, :], w8v[:, :],
                                        axis=mybir.AxisListType.X, op=AOT.add)
                rc = s2.tile([128, 1], f32, tag="rc", name=f"rc_{m}")
                nc.vector.reciprocal(rc[:, :], rs[:, :])
                wo = s2.tile([128, 8], f32, tag="wo", name=f"wo_{m}")
                nc.vector.tensor_scalar(wo[:, :], w8v[:, :], rc[:, 0:1], SCALE,
                                        op0=AOT.mult, op1=AOT.mult)
                nc.sync.dma_start(OW[m * MT:(m + 1) * MT, :], wo[:, :])

            loaded = dict(x_pre)
            loaded[3] = (load_m(3, xh_off, XH_CH), xh_maps(XH_CH))
            groups = [[0, 1, 2]] + [[m] for m in range(3, NM)]
            for group in groups:
                pss = {}
                for gi, m in enumerate(group):
                    pwp, pdp = [(ppw, ppd), (ppw1, ppd1), (ppw2, ppd2)][gi]
                    pw = pwp.tile([128, E], f32, tag=f"pw{gi}", name=f"pw_{m}")
                    pd = pdp.tile([128, E], f32, tag=f"pd{gi}", name=f"pd_{m}")
                    pss[m] = (pw, pd)
                nxt = max(loaded) + 1
                if nxt < NM:
                    loaded[nxt] = (load_m(nxt, xh_off, XH_CH), xh_maps(XH_CH))
                xts = {m: loaded[m] for m in group}
                for k in range(KP):
                    wc, kw = wmap[k]
                    w_ap = w_sbs[wc][:, kw * E:(kw + 1) * E]
                    w8c, kw8 = w8map[k]
                    w8_ap = w8_sbs[w8c][:, kw8 * 2 * E:(kw8 + 1) * 2 * E].rearrange(
                        "p (a e) -> p a e", a=2)
                    for m in group:
                        tiles, xmp = xts[m]
                        xc, kl = xmp[k]
                        xh_t, xd_t = tiles[xc]
                        xh_ap = xh_t[:, kl * MT:(kl + 1) * MT]
                        nkc = (XH_CH0 if m == 0 else XH_CH)[xc]
                        xd_ap = xd_t.rearrange("p (a k t) -> p a k t", a=2,
                                               k=nkc)[:, :, kl, :]
                        pw, pd = pss[m]
                        nc.tensor.matmul(pw[:, :], xh_ap, w_ap,
                                         start=(k == 0), stop=(k == KP - 1))
                        nc.tensor.matmul(pd[:, :], xd_ap, w8_ap,
                                         start=(k == 0), stop=(k == KP - 1),
                                         perf_mode=DR)
                for m in group:
                    stage2(m, *pss[m])

    nc.finalize()
    return nc


def _pack_x(x_shard: np.ndarray):
    """[TLOC, H] f32 -> (xh [NM,128,KP,MT] f16, xl8 [NM,128,KP,MT] f8e4)."""
    xT = np.ascontiguousarray(x_shard.T)               # [H, TLOC]
    xh = xT.astype(np.float16)
    xl = (xT - xh.astype(np.float32)) * SX8
    xh_t = np.zeros((NM, 128, KP, MT), np.float16)
    xh_t[:, :, :KT, :] = xh.reshape(KT, 128, NM, MT).transpose(2, 1, 0, 3)
    xh_t[:, 0, KT, :] = 1.0                            # bias row (-> 32.0 in fp8)
    xl8 = np.zeros((NM, 128, KP, MT), NPF8)
    xl8[:, :, :KT, :] = xl.astype(NPF8).reshape(KT, 128, NM, MT).transpose(2, 1, 0, 3)
    return np.ascontiguousarray(xh_t), np.ascontiguousarray(xl8)


def _pack_w(W: np.ndarray, b: np.ndarray):
    """[E,H] f32 -> (wh [128,KP,E] f16 + b16 row,
                     wdr [128,KP,2E] f8e4 = [w*2^11 | wl*2^22] + b_res row)."""
    wT = np.ascontiguousarray(W.T)                     # [H, E]
    wh = wT.astype(np.float16)
    wl = wT - wh.astype(np.float32)
    whp = np.zeros((128, KP, E), np.float16)
    whp[:, :KT, :] = wh.reshape(KT, 128, E).transpose(1, 0, 2)
    b16 = b.astype(np.float16)
    whp[0, KT, :] = b16
    wdr = np.zeros((128, KP, 2 * E), NPF8)
    wdr[:, :KT, :E] = (wT * SW8).astype(NPF8).reshape(KT, 128, E).transpose(1, 0, 2)
    wdr[:, :KT, E:] = (wl * SWL8).astype(NPF8).reshape(KT, 128, E).transpose(1, 0, 2)
    # slot1 bias residual row: pairs with the 32.0 stationary -> b_res*2^27
    wdr[0, KT, E:] = ((b - b16.astype(np.float32)) * SWL8).astype(NPF8)
    return np.ascontiguousarray(whp), np.ascontiguousarray(wdr)


def prepare_in_maps(hidden_states, W, b, e_score_correction_bias):
    whp, wdr = _pack_w(np.asarray(W, np.float32), np.asarray(b, np.float32))
    consts = np.empty((128, 2 * E), np.float32)
    consts[:, 0:E] = np.asarray(b, np.float32)[None, :]
    consts[:, E:2 * E] = np.asarray(e_score_correction_bias, np.float32)[None, :]
    hs = np.asarray(hidden_states, np.float32)
    in_maps = []
    for c in range(NCORES):
        xh_t, xl8 = _pack_x(hs[c * TLOC:(c + 1) * TLOC])
        in_maps.append({"xh": xh_t, "xl": xl8, "wh": whp, "wdr": wdr,
                        "c": consts})
    return in_maps


def get_prog():
    global _PROG
    if _PROG is None:
        _PROG = _build()
    return _PROG


def kernel(hidden_states, W, b, e_score_correction_bias):
    nc = get_prog()
    in_maps = prepare_in_maps(hidden_states, W, b, e_score_correction_bias)
    res = bass_utils.run_bass_kernel_spmd(nc, in_maps, core_ids=list(range(NCORES)))
    idx = np.concatenate([res.results[c]["oidx"] for c in range(NCORES)], axis=0)
    wts = np.concatenate([res.results[c]["ow"] for c in range(NCORES)], axis=0)
    return idx.astype(np.int32), wts.astype(np.float32)


# revision 25
# speedup vs baseline: 1.0290x; 1.0290x over previous
"""DeepseekV3 top-k router kernel for 8x Trainium2 NeuronCores.

Strategy (v3):
  - Token dim (8192) sharded 8 ways; router weight replicated per core.
  - logits = hidden @ W.T decomposed as  xh*wh  +  (xl*w + xh*wl):
      * P1: one 256-wide fp16 matmul per k-tile: stationary xh16, moving wh16
        -> psum_w.
      * cross: one fp8e4 DoubleRow matmul per k-tile with stationary
        [xl*2^16 | x*2^5] and moving [w*2^11 | (w-wh16)*2^22]; both slot
        products land at scale 2^27 in one psum_d.
    Bias folded exactly via a 57th k-tile (b16 through P1; b_res*2^22 through
    the DoubleRow slot1 against a 32.0 stationary row).
    The fp8 copy of x (slot1) is derived ON-CHIP by the ACT engine from the
    already-resident xh16 tiles (scale-32 copy), so x ships as just
    fp16 hi (2B) + fp8 lo (1B).
    Empirically (fixed-seed inputs) this flips 4 near-tie rows of 8192
    (idx frobenius ~8e-3, weight frobenius ~2e-4) - well inside the gate.
  - stage2 per 128-token tile on DVE/ACT: psum combine, sigmoid, grouped
    top-2-sum via reduce_max + match_replace + reduce_max, top-4 groups,
    masked top-8 via max_with_indices, eb-bias un-correction via one-hot
    match rows + one reduction, normalization.
  - DMA descriptors spread across sync/scalar/gpsimd queues.
"""

import numpy as np
import ml_dtypes

import concourse.bacc as bacc
import concourse.mybir as mybir
from concourse.tile import TileContext
from concourse import bass_utils

H = 7168
E = 256
T = 8192
NCORES = 8
TLOC = T // NCORES          # 1024 tokens per core
MT = 128                    # tokens per tile (PSUM partition dim)
NM = TLOC // MT             # 8 token tiles per core
KT = H // 128               # 56 contraction tiles
KP = KT + 1                 # +1 bias tile
TOP_K = 8
N_GROUP = 8
TOPK_GROUP = 4
EG = E // N_GROUP           # 32 experts per group
SCALE = 2.5
SX8 = 2.0 ** 16             # xl -> fp8 pre-scale
SW8 = 2.0 ** 11             # w  -> fp8 pre-scale
SXH8 = 2.0 ** 5             # x  -> fp8 pre-scale (slot1 stationary)
SWL8 = 2.0 ** 22            # wl -> fp8 pre-scale (slot1 moving)
DSC = 1.0 / (SX8 * SW8)     # = 2^-27; slot1 product matches: 2^5 * 2^22
NEG = -1e30

f32 = mybir.dt.float32
f16 = mybir.dt.float16
f8 = mybir.dt.float8e4
u32 = mybir.dt.uint32
i32 = mybir.dt.int32
AOT = mybir.AluOpType
ACTF = mybir.ActivationFunctionType
DR = mybir.MatmulPerfMode.DoubleRow
NPF8 = ml_dtypes.float8_e4m3

_PROG = None

# per-m chunk sizes in k-tiles (sum = KP = 57); shared by xh and xdr
XH_CH0 = [2, 6, 8, 8, 8, 8, 8, 9]     # m == 0 (fine-grained startup)
XH_CH = [4, 4, 8, 8, 8, 8, 8, 9]      # m > 0
# fp16 wh chunk sizes (k-tiles, sum = 57)
W_CH = [2, 2, 4, 4, 8, 8, 8, 8, 8, 5]
# fp8 wdr chunk sizes (k-tiles, sum = 57)
W8_CH = [2, 2, 4, 4, 8, 8, 8, 8, 8, 5]


def _offs(ch):
    return [sum(ch[:i]) for i in range(len(ch))]


def _build():
    nc = bacc.Bacc(trn_type="TRN2")
    XHD = nc.dram_tensor("xh", [NM, 128, KP, MT], f16, kind="ExternalInput")
    XLD = nc.dram_tensor("xl", [NM, 128, KP, MT], f8, kind="ExternalInput")
    WHD = nc.dram_tensor("wh", [128, KP, E], f16, kind="ExternalInput")
    WDR = nc.dram_tensor("wdr", [128, KP, 2 * E], f8, kind="ExternalInput")
    C = nc.dram_tensor("c", [128, 2 * E], f32, kind="ExternalInput")
    OIDX = nc.dram_tensor("oidx", [TLOC, TOP_K], i32, kind="ExternalOutput")
    OW = nc.dram_tensor("ow", [TLOC, TOP_K], f32, kind="ExternalOutput")

    with TileContext(nc) as tc:
        with (
            tc.tile_pool(name="const", bufs=1) as cpool,
            tc.tile_pool(name="xh0", bufs=1) as xh0pool,
            tc.tile_pool(name="xd0", bufs=1) as xd0pool,
            tc.tile_pool(name="xh", bufs=3) as xhpool,
            tc.tile_pool(name="xd", bufs=3) as xdpool,
            tc.tile_pool(name="s2", bufs=2) as s2,
            tc.tile_pool(name="scrp", bufs=2) as scrp,
            tc.tile_pool(name="pw", bufs=2, space="PSUM") as ppw,
            tc.tile_pool(name="pd", bufs=2, space="PSUM") as ppd,
            tc.tile_pool(name="pw1", bufs=1, space="PSUM") as ppw1,
            tc.tile_pool(name="pd1", bufs=1, space="PSUM") as ppd1,
        ):
            # ---- resident constants / weights (scalar-engine DMA queue) ----
            c_sb = cpool.tile([128, 2 * E], f32, name="c_sb")
            nc.sync.dma_start(c_sb[:, :], C[:, :])
            eb_rep = c_sb[:, E:2 * E]

            w_off = _offs(W_CH)
            w8_off = _offs(W8_CH)
            w_sbs = [None] * len(W_CH)
            w8_sbs = [None] * len(W8_CH)
            merged = []
            wi = w8i = 0
            for k in range(KP):
                while wi < len(W_CH) and w_off[wi] <= k:
                    merged.append(("w", wi)); wi += 1
                while w8i < len(W8_CH) and w8_off[w8i] <= k:
                    merged.append(("w8", w8i)); w8i += 1
            for kind, i in merged:
                if kind == "w":
                    nk = W_CH[i]
                    t = cpool.tile([128, nk * E], f16, name=f"w_sb{i}")
                    nc.sync.dma_start(
                        t.rearrange("p (k e) -> p k e", k=nk),
                        WHD[:, w_off[i]:w_off[i] + nk, :])
                    w_sbs[i] = t
                else:
                    nk = W8_CH[i]
                    t = cpool.tile([128, nk * 2 * E], f8, name=f"w8_sb{i}")
                    nc.sync.dma_start(
                        t.rearrange("p (k e) -> p k e", k=nk),
                        WDR[:, w8_off[i]:w8_off[i] + nk, :])
                    w8_sbs[i] = t

            wmap = []
            for ci, n in enumerate(W_CH):
                wmap += [(ci, j) for j in range(n)]
            w8map = []
            for ci, n in enumerate(W8_CH):
                w8map += [(ci, j) for j in range(n)]

            # ---- x loads: xh fp16 (sync q) + xdr fp8 assembled from a
            #      strided DMA (slot0=xl8, gpsimd q) and an ACT cast of the
            #      xh tile into slot1 (= x*2^5 in fp8) ----
            def load_m(m, offs, ch):
                """Issue all of tile-m's DMAs first, then the fp8 casts, so
                descriptor issue is never blocked behind a data wait."""
                hp = xh0pool if m == 0 else xhpool
                dp = xd0pool if m == 0 else xdpool
                tiles = []
                for i in range(len(ch)):
                    nk = ch[i]
                    xh_t = hp.tile([128, nk * MT], f16, tag=f"xh{m == 0}{i}",
                                   name=f"xh{i}_{m}")
                    nc.scalar.dma_start(
                        xh_t.rearrange("p (k t) -> p k t", k=nk),
                        XHD[m, :, offs[i]:offs[i] + nk, :])
                    xd_t = dp.tile([128, 2 * nk * MT], f8, tag=f"xd{m == 0}{i}",
                                   name=f"xd{i}_{m}")
                    xd4 = xd_t.rearrange("p (a k t) -> p a k t", a=2, k=nk)
                    nc.gpsimd.dma_start(xd4[:, 0, :, :],
                                        XLD[m, :, offs[i]:offs[i] + nk, :])
                    tiles.append((xh_t, xd_t))
                for i in range(len(ch)):
                    nk = ch[i]
                    xh_t, xd_t = tiles[i]
                    xd4 = xd_t.rearrange("p (a k t) -> p a k t", a=2, k=nk)
                    nc.scalar.activation(
                        xd4[:, 1, :, :],
                        xh_t.rearrange("p (k t) -> p k t", k=nk),
                        ACTF.Copy, scale=SXH8)
                return tiles

            xh0_off, xh_off = _offs(XH_CH0), _offs(XH_CH)

            def xh_maps(ch):
                mp = []
                for ci, n in enumerate(ch):
                    mp += [(ci, j) for j in range(n)]
                return mp

            x_pre = {}
            for m, (cho, cof) in ((0, (XH_CH0, xh0_off)), (1, (XH_CH, xh_off)),
                                  (2, (XH_CH, xh_off))):
                x_pre[m] = (load_m(m, cof, cho), xh_maps(cho))

            def stage2(m, pw, pd):
                t0 = s2.tile([128, E], f32, tag="t0", name=f"t0_{m}")
                nc.vector.tensor_scalar(t0[:, :], pd[:, :], DSC, None,
                                        op0=AOT.mult)
                lg = s2.tile([128, E], f32, tag="lg", name=f"lg_{m}")
                nc.vector.scalar_tensor_tensor(
                    lg[:, :], pw[:, :], 1.0, t0[:, :],
                    op0=AOT.mult, op1=AOT.add)
                s = s2.tile([128, E], f32, tag="s", name=f"s_{m}")
                nc.scalar.activation(s[:, :], lg[:, :], ACTF.Sigmoid)
                sfc = s2.tile([128, E], f32, tag="sfc", name=f"sfc_{m}")
                nc.gpsimd.tensor_add(sfc[:, :], s[:, :], eb_rep)

                sfc3 = sfc.rearrange("p (g c) -> p g c", c=EG)
                r1 = s2.tile([128, N_GROUP], f32, tag="r1", name=f"r1_{m}")
                nc.vector.tensor_reduce(r1[:, :], sfc3,
                                        axis=mybir.AxisListType.X, op=AOT.max)
                mr = s2.tile([128, E], f32, tag="mr", name=f"mr_{m}")
                nc.vector.match_replace(mr[:, :], r1[:, :], sfc[:, :], NEG)
                r2 = s2.tile([128, N_GROUP], f32, tag="r2", name=f"r2_{m}")
                nc.vector.tensor_reduce(r2[:, :],
                                        mr.rearrange("p (g c) -> p g c", c=EG),
                                        axis=mybir.AxisListType.X, op=AOT.max)
                gs = s2.tile([128, N_GROUP], f32, tag="gs", name=f"gs_{m}")
                nc.gpsimd.tensor_add(gs[:, :], r1[:, :], r2[:, :])
                g8 = s2.tile([128, 8], f32, tag="g8", name=f"g8_{m}")
                nc.vector.max(out=g8[:, :], in_=gs[:, :])
                gmask = s2.tile([128, N_GROUP], f32, tag="gmask", name=f"gmask_{m}")
                nc.vector.tensor_scalar(gmask[:, :], gs[:, :],
                                        g8[:, TOPK_GROUP - 1:TOPK_GROUP], None,
                                        op0=AOT.is_ge)
                masked = s2.tile([128, E], f32, tag="masked", name=f"masked_{m}")
                nc.gpsimd.tensor_mul(
                    masked.rearrange("p (g c) -> p g c", c=EG), sfc3,
                    gmask.unsqueeze(2).to_broadcast([128, N_GROUP, EG]))
                m8 = s2.tile([128, 8], f32, tag="m8", name=f"m8_{m}")
                i8 = s2.tile([128, 8], u32, tag="i8", name=f"i8_{m}")
                nc.vector.max_with_indices(out_max=m8[:, :], out_indices=i8[:, :],
                                           in_=masked[:, :])
                nc.sync.dma_start(OIDX[m * MT:(m + 1) * MT, :], i8.bitcast(i32))

                # eb[idx_j] via one-hot match rows; halves split across the
                # Vector and Pool engines to halve the chain latency
                eb8 = s2.tile([128, 8], f32, tag="eb8", name=f"eb8_{m}")
                scr = scrp.tile([128, TOP_K * E], f16, tag="scr", name=f"scr_{m}")
                for j in range(TOP_K):
                    nc.vector.scalar_tensor_tensor(
                        scr[:, j * E:(j + 1) * E], sfc[:, :], m8[:, j:j + 1],
                        eb_rep, op0=AOT.is_equal, op1=AOT.mult)
                # fp16 binary add-tree on the Pool engine: 256 -> 1 per row
                ta = scrp.tile([128, TOP_K * 128], f16, tag="ta", name=f"ta_{m}")
                tb = scrp.tile([128, TOP_K * 64], f16, tag="tb", name=f"tb_{m}")
                w = E
                cur = scr
                dsts = [ta, tb, ta, tb, ta, tb, ta]
                for lvl in range(7):
                    w //= 2
                    dst = dsts[lvl]
                    c3 = cur.rearrange("p (j e) -> p j e", j=TOP_K)
                    d3 = dst.rearrange("p (j e) -> p j e", j=TOP_K)                         if dst.shape[1] == TOP_K * w else                         dst.rearrange("p (j e) -> p j e", j=TOP_K)[:, :, :w]
                    nc.gpsimd.tensor_add(d3[:, :, 0:w], c3[:, :, 0:w],
                                         c3[:, :, w:2 * w])
                    cur = dst
                c3 = cur.rearrange("p (j e) -> p j e", j=TOP_K)
                nc.gpsimd.tensor_add(eb8.unsqueeze(2), c3[:, :, 0:1],
                                     c3[:, :, 1:2])
                w8v = s2.tile([128, 8], f32, tag="w8v", name=f"w8v_{m}")
                nc.vector.tensor_tensor(w8v[:, :], m8[:, :], eb8[:, :],
                                        op=AOT.subtract)
                rs = s2.tile([128, 1], f32, tag="rs", name=f"rs_{m}")
                nc.vector.tensor_reduce(rs[:, :], w8v[:, :],
                                        axis=mybir.AxisListType.X, op=AOT.add)
                rc = s2.tile([128, 1], f32, tag="rc", name=f"rc_{m}")
                nc.vector.reciprocal(rc[:, :], rs[:, :])
                wo = s2.tile([128, 8], f32, tag="wo", name=f"wo_{m}")
                nc.vector.tensor_scalar(wo[:, :], w8v[:, :], rc[:, 0:1], SCALE,
                                        op0=AOT.mult, op1=AOT.mult)
                nc.sync.dma_start(OW[m * MT:(m + 1) * MT, :], wo[:, :])

            loaded = dict(x_pre)
            loaded[3] = (load_m(3, xh_off, XH_CH), xh_maps(XH_CH))
            groups = [[0, 1]] + [[m] for m in range(2, NM)]
            for group in groups:
                pss = {}
                for gi, m in enumerate(group):
                    pwp, pdp = [(ppw, ppd), (ppw1, ppd1)][gi]
                    pw = pwp.tile([128, E], f32, tag=f"pw{gi}", name=f"pw_{m}")
                    pd = pdp.tile([128, E], f32, tag=f"pd{gi}", name=f"pd_{m}")
                    pss[m] = (pw, pd)
                nxt = max(loaded) + 1
                if nxt < NM:
                    loaded[nxt] = (load_m(nxt, xh_off, XH_CH), xh_maps(XH_CH))
                xts = {m: loaded[m] for m in group}
                for k in range(KP):
                    wc, kw = wmap[k]
                    w_ap = w_sbs[wc][:, kw * E:(kw + 1) * E]
                    w8c, kw8 = w8map[k]
                    w8_ap = w8_sbs[w8c][:, kw8 * 2 * E:(kw8 + 1) * 2 * E].rearrange(
                        "p (a e) -> p a e", a=2)
                    for m in group:
                        tiles, xmp = xts[m]
                        xc, kl = xmp[k]
                        xh_t, xd_t = tiles[xc]
                        xh_ap = xh_t[:, kl * MT:(kl + 1) * MT]
                        nkc = (XH_CH0 if m == 0 else XH_CH)[xc]
                        xd_ap = xd_t.rearrange("p (a k t) -> p a k t", a=2,
                                               k=nkc)[:, :, kl, :]
                        pw, pd = pss[m]
                        nc.tensor.matmul(pw[:, :], xh_ap, w_ap,
                                         start=(k == 0), stop=(k == KP - 1))
                        nc.tensor.matmul(pd[:, :], xd_ap, w8_ap,
                                         start=(k == 0), stop=(k == KP - 1),
                                         perf_mode=DR)
                for m in group:
                    stage2(m, *pss[m])

    nc.finalize()
    return nc


def _pack_x(x_shard: np.ndarray):
    """[TLOC, H] f32 -> (xh [NM,128,KP,MT] f16, xl8 [NM,128,KP,MT] f8e4)."""
    xT = np.ascontiguousarray(x_shard.T)               # [H, TLOC]
    xh = xT.astype(np.float16)
    xl = (xT - xh.astype(np.float32)) * SX8
    xh_t = np.zeros((NM, 128, KP, MT), np.float16)
    xh_t[:, :, :KT, :] = xh.reshape(KT, 128, NM, MT).transpose(2, 1, 0, 3)
    xh_t[:, 0, KT, :] = 1.0                            # bias row (-> 32.0 in fp8)
    xl8 = np.zeros((NM, 128, KP, MT), NPF8)
    xl8[:, :, :KT, :] = xl.astype(NPF8).reshape(KT, 128, NM, MT).transpose(2, 1, 0, 3)
    return np.ascontiguousarray(xh_t), np.ascontiguousarray(xl8)


def _pack_w(W: np.ndarray, b: np.ndarray):
    """[E,H] f32 -> (wh [128,KP,E] f16 + b16 row,
                     wdr [128,KP,2E] f8e4 = [w*2^11 | wl*2^22] + b_res row)."""
    wT = np.ascontiguousarray(W.T)                     # [H, E]
    wh = wT.astype(np.float16)
    wl = wT - wh.astype(np.float32)
    whp = np.zeros((128, KP, E), np.float16)
    whp[:, :KT, :] = wh.reshape(KT, 128, E).transpose(1, 0, 2)
    b16 = b.astype(np.float16)
    whp[0, KT, :] = b16
    wdr = np.zeros((128, KP, 2 * E), NPF8)
    wdr[:, :KT, :E] = (wT * SW8).astype(NPF8).reshape(KT, 128, E).transpose(1, 0, 2)
    wdr[:, :KT, E:] = (wl * SWL8).astype(NPF8).reshape(KT, 128, E).transpose(1, 0, 2)
    # slot1 bias residual row: pairs with the 32.0 stationary -> b_res*2^27
    wdr[0, KT, E:] = ((b - b16.astype(np.float32)) * SWL8).astype(NPF8)
    return np.ascontiguousarray(whp), np.ascontiguousarray(wdr)


def prepare_in_maps(hidden_states, W, b, e_score_correction_bias):
    whp, wdr = _pack_w(np.asarray(W, np.float32), np.asarray(b, np.float32))
    consts = np.empty((128, 2 * E), np.float32)
    consts[:, 0:E] = np.asarray(b, np.float32)[None, :]
    consts[:, E:2 * E] = np.asarray(e_score_correction_bias, np.float32)[None, :]
    hs = np.asarray(hidden_states, np.float32)
    in_maps = []
    for c in range(NCORES):
        xh_t, xl8 = _pack_x(hs[c * TLOC:(c + 1) * TLOC])
        in_maps.append({"xh": xh_t, "xl": xl8, "wh": whp, "wdr": wdr,
                        "c": consts})
    return in_maps


def get_prog():
    global _PROG
    if _PROG is None:
        _PROG = _build()
    return _PROG


def kernel(hidden_states, W, b, e_score_correction_bias):
    nc = get_prog()
    in_maps = prepare_in_maps(hidden_states, W, b, e_score_correction_bias)
    res = bass_utils.run_bass_kernel_spmd(nc, in_maps, core_ids=list(range(NCORES)))
    idx = np.concatenate([res.results[c]["oidx"] for c in range(NCORES)], axis=0)
    wts = np.concatenate([res.results[c]["ow"] for c in range(NCORES)], axis=0)
    return idx.astype(np.int32), wts.astype(np.float32)


# revision 27
# speedup vs baseline: 1.0506x; 1.0210x over previous
"""DeepseekV3 top-k router kernel for 8x Trainium2 NeuronCores.

Strategy (v3):
  - Token dim (8192) sharded 8 ways; router weight replicated per core.
  - logits = hidden @ W.T decomposed as  xh*wh  +  (xl*w + xh*wl):
      * P1: one 256-wide fp16 matmul per k-tile: stationary xh16, moving wh16
        -> psum_w.
      * cross: one fp8e4 DoubleRow matmul per k-tile with stationary
        [xl*2^16 | x*2^5] and moving [w*2^11 | (w-wh16)*2^22]; both slot
        products land at scale 2^27 in one psum_d.
    Bias folded exactly via a 57th k-tile (b16 through P1; b_res*2^22 through
    the DoubleRow slot1 against a 32.0 stationary row).
    The fp8 copy of x (slot1) is derived ON-CHIP by the ACT engine from the
    already-resident xh16 tiles (scale-32 copy), so x ships as just
    fp16 hi (2B) + fp8 lo (1B).
    Empirically (fixed-seed inputs) this flips 4 near-tie rows of 8192
    (idx frobenius ~8e-3, weight frobenius ~2e-4) - well inside the gate.
  - stage2 per 128-token tile on DVE/ACT: psum combine, sigmoid, grouped
    top-2-sum via reduce_max + match_replace + reduce_max, top-4 groups,
    masked top-8 via max_with_indices, eb-bias un-correction via one-hot
    match rows + one reduction, normalization.
  - DMA descriptors spread across sync/scalar/gpsimd queues.
"""

import numpy as np
import ml_dtypes

import concourse.bacc as bacc
import concourse.mybir as mybir
from concourse.tile import TileContext
from concourse import bass_utils

H = 7168
E = 256
T = 8192
NCORES = 8
TLOC = T // NCORES          # 1024 tokens per core
MT = 128                    # tokens per tile (PSUM partition dim)
NM = TLOC // MT             # 8 token tiles per core
KT = H // 128               # 56 contraction tiles
KP = KT + 1                 # +1 bias tile
TOP_K = 8
N_GROUP = 8
TOPK_GROUP = 4
EG = E // N_GROUP           # 32 experts per group
SCALE = 2.5
SX8 = 2.0 ** 16             # xl -> fp8 pre-scale
SW8 = 2.0 ** 11             # w  -> fp8 pre-scale
SXH8 = 2.0 ** 5             # x  -> fp8 pre-scale (slot1 stationary)
SWL8 = 2.0 ** 22            # wl -> fp8 pre-scale (slot1 moving)
DSC = 1.0 / (SX8 * SW8)     # = 2^-27; slot1 product matches: 2^5 * 2^22
NEG = -1e30

f32 = mybir.dt.float32
f16 = mybir.dt.float16
f8 = mybir.dt.float8e4
u32 = mybir.dt.uint32
i32 = mybir.dt.int32
AOT = mybir.AluOpType
ACTF = mybir.ActivationFunctionType
DR = mybir.MatmulPerfMode.DoubleRow
NPF8 = ml_dtypes.float8_e4m3

_PROG = None

# per-m chunk sizes in k-tiles (sum = KP = 57); shared by xh and xdr
XH_CH0 = [2, 6, 8, 8, 8, 8, 8, 9]     # m == 0 (fine-grained startup)
XH_CH = [4, 4, 8, 8, 8, 8, 8, 9]      # m > 0
# fp16 wh chunk sizes (k-tiles, sum = 57)
W_CH = [2, 2, 4, 4, 8, 8, 8, 8, 8, 5]
# fp8 wdr chunk sizes (k-tiles, sum = 57)
W8_CH = [2, 2, 4, 4, 8, 8, 8, 8, 8, 5]


def _offs(ch):
    return [sum(ch[:i]) for i in range(len(ch))]


def _build():
    nc = bacc.Bacc(trn_type="TRN2")
    XHD = nc.dram_tensor("xh", [NM, 128, KP, MT], f16, kind="ExternalInput")
    XLD = nc.dram_tensor("xl", [NM, 128, KP, MT], f8, kind="ExternalInput")
    WHD = nc.dram_tensor("wh", [128, KP, E], f16, kind="ExternalInput")
    WDR = nc.dram_tensor("wdr", [128, KP, 2 * E], f8, kind="ExternalInput")
    C = nc.dram_tensor("c", [128, 2 * E], f32, kind="ExternalInput")
    OIDX = nc.dram_tensor("oidx", [TLOC, TOP_K], i32, kind="ExternalOutput")
    OW = nc.dram_tensor("ow", [TLOC, TOP_K], f32, kind="ExternalOutput")

    with TileContext(nc) as tc:
        with (
            tc.tile_pool(name="const", bufs=1) as cpool,
            tc.tile_pool(name="xh0", bufs=1) as xh0pool,
            tc.tile_pool(name="xd0", bufs=1) as xd0pool,
            tc.tile_pool(name="xh", bufs=3) as xhpool,
            tc.tile_pool(name="xd", bufs=3) as xdpool,
            tc.tile_pool(name="s2", bufs=2) as s2,
            tc.tile_pool(name="scrp", bufs=2) as scrp,
            tc.tile_pool(name="pw", bufs=2, space="PSUM") as ppw,
            tc.tile_pool(name="pd", bufs=2, space="PSUM") as ppd,
            tc.tile_pool(name="pw1", bufs=1, space="PSUM") as ppw1,
            tc.tile_pool(name="pd1", bufs=1, space="PSUM") as ppd1,
        ):
            # ---- resident constants / weights (scalar-engine DMA queue) ----
            c_sb = cpool.tile([128, 2 * E], f32, name="c_sb")
            nc.sync.dma_start(c_sb[:, :], C[:, :])
            eb_rep = c_sb[:, E:2 * E]

            w_off = _offs(W_CH)
            w8_off = _offs(W8_CH)
            w_sbs = [None] * len(W_CH)
            w8_sbs = [None] * len(W8_CH)
            merged = []
            wi = w8i = 0
            for k in range(KP):
                while wi < len(W_CH) and w_off[wi] <= k:
                    merged.append(("w", wi)); wi += 1
                while w8i < len(W8_CH) and w8_off[w8i] <= k:
                    merged.append(("w8", w8i)); w8i += 1
            for kind, i in merged:
                if kind == "w":
                    nk = W_CH[i]
                    t = cpool.tile([128, nk * E], f16, name=f"w_sb{i}")
                    nc.sync.dma_start(
                        t.rearrange("p (k e) -> p k e", k=nk),
                        WHD[:, w_off[i]:w_off[i] + nk, :])
                    w_sbs[i] = t
                else:
                    nk = W8_CH[i]
                    t = cpool.tile([128, nk * 2 * E], f8, name=f"w8_sb{i}")
                    nc.sync.dma_start(
                        t.rearrange("p (k e) -> p k e", k=nk),
                        WDR[:, w8_off[i]:w8_off[i] + nk, :])
                    w8_sbs[i] = t

            wmap = []
            for ci, n in enumerate(W_CH):
                wmap += [(ci, j) for j in range(n)]
            w8map = []
            for ci, n in enumerate(W8_CH):
                w8map += [(ci, j) for j in range(n)]

            # ---- x loads: xh fp16 (sync q) + xdr fp8 assembled from a
            #      strided DMA (slot0=xl8, gpsimd q) and an ACT cast of the
            #      xh tile into slot1 (= x*2^5 in fp8) ----
            def load_dmas(m, offs, ch):
                hp = xh0pool if m == 0 else xhpool
                dp = xd0pool if m == 0 else xdpool
                tiles = []
                for i in range(len(ch)):
                    nk = ch[i]
                    xh_t = hp.tile([128, nk * MT], f16, tag=f"xh{m == 0}{i}",
                                   name=f"xh{i}_{m}")
                    nc.scalar.dma_start(
                        xh_t.rearrange("p (k t) -> p k t", k=nk),
                        XHD[m, :, offs[i]:offs[i] + nk, :])
                    xd_t = dp.tile([128, 2 * nk * MT], f8, tag=f"xd{m == 0}{i}",
                                   name=f"xd{i}_{m}")
                    xd4 = xd_t.rearrange("p (a k t) -> p a k t", a=2, k=nk)
                    nc.gpsimd.dma_start(xd4[:, 0, :, :],
                                        XLD[m, :, offs[i]:offs[i] + nk, :])
                    tiles.append((xh_t, xd_t))
                return tiles

            def load_casts(tiles, ch):
                """fp8 slot1 casts on ACT; emitted AFTER the current group's
                stage2 so sigmoids never queue behind them."""
                for i in range(len(ch)):
                    nk = ch[i]
                    xh_t, xd_t = tiles[i]
                    xd4 = xd_t.rearrange("p (a k t) -> p a k t", a=2, k=nk)
                    nc.scalar.activation(
                        xd4[:, 1, :, :],
                        xh_t.rearrange("p (k t) -> p k t", k=nk),
                        ACTF.Copy, scale=SXH8)

            def load_m(m, offs, ch):
                tiles = load_dmas(m, offs, ch)
                load_casts(tiles, ch)
                return tiles

            xh0_off, xh_off = _offs(XH_CH0), _offs(XH_CH)

            def xh_maps(ch):
                mp = []
                for ci, n in enumerate(ch):
                    mp += [(ci, j) for j in range(n)]
                return mp

            x_pre = {}
            for m, (cho, cof) in ((0, (XH_CH0, xh0_off)), (1, (XH_CH, xh_off)),
                                  (2, (XH_CH, xh_off))):
                x_pre[m] = (load_m(m, cof, cho), xh_maps(cho))

            def stage2(m, pw, pd):
                t0 = s2.tile([128, E], f32, tag="t0", name=f"t0_{m}")
                nc.vector.tensor_scalar(t0[:, :], pd[:, :], DSC, None,
                                        op0=AOT.mult)
                lg = s2.tile([128, E], f32, tag="lg", name=f"lg_{m}")
                nc.vector.scalar_tensor_tensor(
                    lg[:, :], pw[:, :], 1.0, t0[:, :],
                    op0=AOT.mult, op1=AOT.add)
                s = s2.tile([128, E], f32, tag="s", name=f"s_{m}")
                nc.scalar.activation(s[:, :], lg[:, :], ACTF.Sigmoid)
                sfc = s2.tile([128, E], f32, tag="sfc", name=f"sfc_{m}")
                nc.gpsimd.tensor_add(sfc[:, :], s[:, :], eb_rep)

                sfc3 = sfc.rearrange("p (g c) -> p g c", c=EG)
                r1 = s2.tile([128, N_GROUP], f32, tag="r1", name=f"r1_{m}")
                nc.vector.tensor_reduce(r1[:, :], sfc3,
                                        axis=mybir.AxisListType.X, op=AOT.max)
                mr = s2.tile([128, E], f32, tag="mr", name=f"mr_{m}")
                nc.vector.match_replace(mr[:, :], r1[:, :], sfc[:, :], NEG)
                r2 = s2.tile([128, N_GROUP], f32, tag="r2", name=f"r2_{m}")
                nc.vector.tensor_reduce(r2[:, :],
                                        mr.rearrange("p (g c) -> p g c", c=EG),
                                        axis=mybir.AxisListType.X, op=AOT.max)
                gs = s2.tile([128, N_GROUP], f32, tag="gs", name=f"gs_{m}")
                nc.gpsimd.tensor_add(gs[:, :], r1[:, :], r2[:, :])
                g8 = s2.tile([128, 8], f32, tag="g8", name=f"g8_{m}")
                nc.vector.max(out=g8[:, :], in_=gs[:, :])
                gmask = s2.tile([128, N_GROUP], f32, tag="gmask", name=f"gmask_{m}")
                nc.vector.tensor_scalar(gmask[:, :], gs[:, :],
                                        g8[:, TOPK_GROUP - 1:TOPK_GROUP], None,
                                        op0=AOT.is_ge)
                masked = s2.tile([128, E], f32, tag="masked", name=f"masked_{m}")
                nc.gpsimd.tensor_mul(
                    masked.rearrange("p (g c) -> p g c", c=EG), sfc3,
                    gmask.unsqueeze(2).to_broadcast([128, N_GROUP, EG]))
                m8 = s2.tile([128, 8], f32, tag="m8", name=f"m8_{m}")
                i8 = s2.tile([128, 8], u32, tag="i8", name=f"i8_{m}")
                nc.vector.max_with_indices(out_max=m8[:, :], out_indices=i8[:, :],
                                           in_=masked[:, :])
                nc.sync.dma_start(OIDX[m * MT:(m + 1) * MT, :], i8.bitcast(i32))

                # eb[idx_j] via one-hot match rows; halves split across the
                # Vector and Pool engines to halve the chain latency
                eb8 = s2.tile([128, 8], f32, tag="eb8", name=f"eb8_{m}")
                scr = scrp.tile([128, TOP_K * E], f16, tag="scr", name=f"scr_{m}")
                for j in range(TOP_K):
                    nc.vector.scalar_tensor_tensor(
                        scr[:, j * E:(j + 1) * E], sfc[:, :], m8[:, j:j + 1],
                        eb_rep, op0=AOT.is_equal, op1=AOT.mult)
                # fp16 binary add-tree on the Pool engine: 256 -> 1 per row
                ta = scrp.tile([128, TOP_K * 128], f16, tag="ta", name=f"ta_{m}")
                tb = scrp.tile([128, TOP_K * 64], f16, tag="tb", name=f"tb_{m}")
                w = E
                cur = scr
                dsts = [ta, tb, ta, tb, ta, tb, ta]
                for lvl in range(7):
                    w //= 2
                    dst = dsts[lvl]
                    c3 = cur.rearrange("p (j e) -> p j e", j=TOP_K)
                    d3 = dst.rearrange("p (j e) -> p j e", j=TOP_K)                         if dst.shape[1] == TOP_K * w else                         dst.rearrange("p (j e) -> p j e", j=TOP_K)[:, :, :w]
                    nc.gpsimd.tensor_add(d3[:, :, 0:w], c3[:, :, 0:w],
                                         c3[:, :, w:2 * w])
                    cur = dst
                c3 = cur.rearrange("p (j e) -> p j e", j=TOP_K)
                nc.gpsimd.tensor_add(eb8.unsqueeze(2), c3[:, :, 0:1],
                                     c3[:, :, 1:2])
                w8v = s2.tile([128, 8], f32, tag="w8v", name=f"w8v_{m}")
                nc.vector.tensor_tensor(w8v[:, :], m8[:, :], eb8[:, :],
                                        op=AOT.subtract)
                rs = s2.tile([128, 1], f32, tag="rs", name=f"rs_{m}")
                nc.vector.tensor_reduce(rs[:, :], w8v[:, :],
                                        axis=mybir.AxisListType.X, op=AOT.add)
                rc = s2.tile([128, 1], f32, tag="rc", name=f"rc_{m}")
                nc.vector.reciprocal(rc[:, :], rs[:, :])
                wo = s2.tile([128, 8], f32, tag="wo", name=f"wo_{m}")
                nc.vector.tensor_scalar(wo[:, :], w8v[:, :], rc[:, 0:1], SCALE,
                                        op0=AOT.mult, op1=AOT.mult)
                nc.sync.dma_start(OW[m * MT:(m + 1) * MT, :], wo[:, :])

            loaded = dict(x_pre)
            _t3 = load_dmas(3, xh_off, XH_CH)
            loaded[3] = (_t3, xh_maps(XH_CH))
            _t3_pending_casts = [_t3]
            groups = [[0, 1]] + [[m] for m in range(2, NM)]
            for group in groups:
                pss = {}
                for gi, m in enumerate(group):
                    pwp, pdp = [(ppw, ppd), (ppw1, ppd1)][gi]
                    pw = pwp.tile([128, E], f32, tag=f"pw{gi}", name=f"pw_{m}")
                    pd = pdp.tile([128, E], f32, tag=f"pd{gi}", name=f"pd_{m}")
                    pss[m] = (pw, pd)
                nxt = max(loaded) + 1
                pend = None
                if nxt < NM:
                    tl = load_dmas(nxt, xh_off, XH_CH)
                    loaded[nxt] = (tl, xh_maps(XH_CH))
                    pend = tl
                xts = {m: loaded[m] for m in group}
                for k in range(KP):
                    wc, kw = wmap[k]
                    w_ap = w_sbs[wc][:, kw * E:(kw + 1) * E]
                    w8c, kw8 = w8map[k]
                    w8_ap = w8_sbs[w8c][:, kw8 * 2 * E:(kw8 + 1) * 2 * E].rearrange(
                        "p (a e) -> p a e", a=2)
                    for m in group:
                        tiles, xmp = xts[m]
                        xc, kl = xmp[k]
                        xh_t, xd_t = tiles[xc]
                        xh_ap = xh_t[:, kl * MT:(kl + 1) * MT]
                        nkc = (XH_CH0 if m == 0 else XH_CH)[xc]
                        xd_ap = xd_t.rearrange("p (a k t) -> p a k t", a=2,
                                               k=nkc)[:, :, kl, :]
                        pw, pd = pss[m]
                        nc.tensor.matmul(pw[:, :], xh_ap, w_ap,
                                         start=(k == 0), stop=(k == KP - 1))
                        nc.tensor.matmul(pd[:, :], xd_ap, w8_ap,
                                         start=(k == 0), stop=(k == KP - 1),
                                         perf_mode=DR)
                for m in group:
                    stage2(m, *pss[m])
                while _t3_pending_casts:
                    load_casts(_t3_pending_casts.pop(), XH_CH)
                if pend is not None:
                    load_casts(pend, XH_CH)

    nc.finalize()
    return nc


def _pack_x(x_shard: np.ndarray):
    """[TLOC, H] f32 -> (xh [NM,128,KP,MT] f16, xl8 [NM,128,KP,MT] f8e4)."""
    xT = np.ascontiguousarray(x_shard.T)               # [H, TLOC]
    xh = xT.astype(np.float16)
    xl = (xT - xh.astype(np.float32)) * SX8
    xh_t = np.zeros((NM, 128, KP, MT), np.float16)
    xh_t[:, :, :KT, :] = xh.reshape(KT, 128, NM, MT).transpose(2, 1, 0, 3)
    xh_t[:, 0, KT, :] = 1.0                            # bias row (-> 32.0 in fp8)
    xl8 = np.zeros((NM, 128, KP, MT), NPF8)
    xl8[:, :, :KT, :] = xl.astype(NPF8).reshape(KT, 128, NM, MT).transpose(2, 1, 0, 3)
    return np.ascontiguousarray(xh_t), np.ascontiguousarray(xl8)


def _pack_w(W: np.ndarray, b: np.ndarray):
    """[E,H] f32 -> (wh [128,KP,E] f16 + b16 row,
                     wdr [128,KP,2E] f8e4 = [w*2^11 | wl*2^22] + b_res row)."""
    wT = np.ascontiguousarray(W.T)                     # [H, E]
    wh = wT.astype(np.float16)
    wl = wT - wh.astype(np.float32)
    whp = np.zeros((128, KP, E), np.float16)
    whp[:, :KT, :] = wh.reshape(KT, 128, E).transpose(1, 0, 2)
    b16 = b.astype(np.float16)
    whp[0, KT, :] = b16
    wdr = np.zeros((128, KP, 2 * E), NPF8)
    wdr[:, :KT, :E] = (wT * SW8).astype(NPF8).reshape(KT, 128, E).transpose(1, 0, 2)
    wdr[:, :KT, E:] = (wl * SWL8).astype(NPF8).reshape(KT, 128, E).transpose(1, 0, 2)
    # slot1 bias residual row: pairs with the 32.0 stationary -> b_res*2^27
    wdr[0, KT, E:] = ((b - b16.astype(np.float32)) * SWL8).astype(NPF8)
    return np.ascontiguousarray(whp), np.ascontiguousarray(wdr)


def prepare_in_maps(hidden_states, W, b, e_score_correction_bias):
    whp, wdr = _pack_w(np.asarray(W, np.float32), np.asarray(b, np.float32))
    consts = np.empty((128, 2 * E), np.float32)
    consts[:, 0:E] = np.asarray(b, np.float32)[None, :]
    consts[:, E:2 * E] = np.asarray(e_score_correction_bias, np.float32)[None, :]
    hs = np.asarray(hidden_states, np.float32)
    in_maps = []
    for c in range(NCORES):
        xh_t, xl8 = _pack_x(hs[c * TLOC:(c + 1) * TLOC])
        in_maps.append({"xh": xh_t, "xl": xl8, "wh": whp, "wdr": wdr,
                        "c": consts})
    return in_maps


def get_prog():
    global _PROG
    if _PROG is None:
        _PROG = _build()
    return _PROG


def kernel(hidden_states, W, b, e_score_correction_bias):
    nc = get_prog()
    in_maps = prepare_in_maps(hidden_states, W, b, e_score_correction_bias)
    res = bass_utils.run_bass_kernel_spmd(nc, in_maps, core_ids=list(range(NCORES)))
    idx = np.concatenate([res.results[c]["oidx"] for c in range(NCORES)], axis=0)
    wts = np.concatenate([res.results[c]["ow"] for c in range(NCORES)], axis=0)
    return idx.astype(np.int32), wts.astype(np.float32)
